# revision 39
# baseline (speedup 1.0000x reference)
"""GPT-2 forward on 8 TRN2 NeuronCores — strided context-parallel Bass/Tile kernel.

Sharding: 4 sequences x 2 cores each. Core 2b+p owns tokens of sequence b at
global positions {2u+p : u in [0, S/2)}.

v2 design (vs the DMA-transpose baseline):
- The residual h lives FEATURE-major ([E-part, T]) for the whole network, so
  no transposes exist anywhere. LN stats (sum, sumsq) are computed on the PE
  with ones-vector matmuls (fp32r rhs for the raw sum at full speed, bf16 h^2
  from an ACT Square for the sumsq); per-token mean/rstd rows are broadcast
  across partitions with 1-partition PE matmuls and applied with one GPSIMD
  subtract + one DVE multiply per chunk.
- Per layer the pair exchanges the LN1 OUTPUT a (bf16, 786KB) via AllGather
  issued at the END of the previous layer (right after the fc2 residual), so
  the collective flies during ln1/qkv. Each core then computes k,v for BOTH
  parities locally (weights are shared), which keeps all PSUM->SBUF k/v
  copies partition-identity.
- Attention keys are PARITY-GROUPED: chunk (par, c) holds parity-par tokens
  [128c, 128c+128); both par-chunks at c cover queries [128c, T) with a
  single 128x128 diagonal mask each (own: inclusive tril; peer: strict tril
  for even cores, inclusive for odd). Scores keys-major, softmax without max
  subtraction, exp on ACT, av with a ones-column in v so denominators ride in
  the same matmul, per-query normalization via approx-reciprocal + rank-1 PE
  broadcast. The first few heads run their own-parity pass before the peer
  data lands to keep the PE warm across the collective.
- lm_head streams 512-vocab-column chunks with double-buffered weight loads;
  logits are stored bf16 and widened on the host.
"""
import sys, os
sys.path.insert(0, '/opt/trn_rl_repo')
import numpy as np
import ml_dtypes
import concourse.bass as bass
import concourse.mybir as mybir
from concourse import bacc
from concourse.bass_utils import run_bass_kernel_spmd
from concourse.tile import TileContext

F32 = mybir.dt.float32
F32R = mybir.dt.float32r
BF16 = mybir.dt.bfloat16
AF = mybir.ActivationFunctionType
ALU = mybir.AluOpType
BF = ml_dtypes.bfloat16
EPS = 1e-5


def cfg_full():
    return dict(B=4, S=1024, L=12, H=12, D=64, F=3072, V=50257)


def cfg_mini():
    return dict(B=4, S=256, L=2, H=2, D=64, F=256, V=640)


def derived(c):
    d = dict(c)
    d['E'] = c['H'] * c['D']
    d['T'] = c['S'] // 2          # local tokens per core
    d['CH'] = d['T'] // 128       # 128-token chunks per parity
    d['ECH'] = d['E'] // 128
    d['FCH'] = c['F'] // 128
    d['VNC'] = (c['V'] + 511) // 512
    d['NW'] = d['E'] // 2         # v column half
    d['H2'] = c['H'] // 2         # heads per v half
    assert d['T'] % 128 == 0 and d['E'] % 128 == 0 and c['F'] % 128 == 0
    assert c['H'] % 2 == 0 and c['D'] == 64
    return d


def build(c, has_bias, dump=()):
    d = derived(c)
    T, E, H, D, F, V, L = d['T'], d['E'], c['H'], c['D'], c['F'], c['V'], c['L']
    CH, ECH, FCH, VNC, NW, H2 = d['CH'], d['ECH'], d['FCH'], d['VNC'], d['NW'], d['H2']

    nc = bacc.Bacc("TRN2", target_bir_lowering=False, debug=False, num_devices=8)

    h0_p = nc.declare_dram_parameter("h0", [E, T], F32, isOutput=False)
    wqkv_p = nc.declare_dram_parameter("wqkv", [L, 128, ECH, 3 * E], BF16, isOutput=False)
    wproj_p = nc.declare_dram_parameter("wproj", [L, 128, ECH, E], BF16, isOutput=False)
    wfc_p = nc.declare_dram_parameter("wfc", [L, 128, ECH, F], BF16, isOutput=False)
    wfc2_p = nc.declare_dram_parameter("wfc2", [L, 128, FCH, E], BF16, isOutput=False)
    wlm_p = nc.declare_dram_parameter("wlm", [128, ECH, VNC * 512], BF16, isOutput=False)
    masks_p = nc.declare_dram_parameter("masks", [2, 128, 128], BF16, isOutput=False)
    if has_bias['qkv']:
        bqk_p = nc.declare_dram_parameter("bqk", [L, 2 * ECH, 128, 1], F32, isOutput=False)
    if has_bias['v']:
        bv_p = nc.declare_dram_parameter("bv", [L, 1, E], BF16, isOutput=False)
    if has_bias['proj']:
        bproj_p = nc.declare_dram_parameter("bproj", [L, ECH, 128, 1], F32, isOutput=False)
    if has_bias['fc']:
        bfc_p = nc.declare_dram_parameter("bfc", [L, FCH, 128, 1], F32, isOutput=False)
    if has_bias['fc2']:
        bfc2_p = nc.declare_dram_parameter("bfc2", [L, ECH, 128, 1], F32, isOutput=False)
    if has_bias['lm']:
        blm_p = nc.declare_dram_parameter("blm", [1, VNC * 512], BF16, isOutput=False)
    out_p = nc.declare_dram_parameter("logits", [T, VNC * 512], BF16, isOutput=True)

    dump = set(dump)
    dump_p = {nm: nc.declare_dram_parameter("d_" + nm, shp, dt, isOutput=True)
              for nm, shp, dt in [
                  ("aT", [128, ECH * T], BF16), ("qT", [128, ECH * T], BF16),
                  ("kTo", [128, ECH * T], BF16), ("kTp", [128, ECH * T], BF16),
                  ("vo", [128, CH * H * 65], BF16), ("vp", [128, CH * H * 65], BF16),
                  ("yTc", [128, ECH * T], BF16), ("h1", [128, ECH * T], F32),
                  ("gT", [128, FCH * T], BF16), ("h2", [128, ECH * T], F32)]
              if nm in dump}

    def do_dump(nm, tile):
        if nm in dump:
            nc.sync.dma_start(dump_p[nm].ap(), tile[:].rearrange(
                " ".join(["p"] + [chr(97 + i) for i in range(len(tile.shape) - 1)])
                + " -> p (" + " ".join([chr(97 + i) for i in range(len(tile.shape) - 1)]) + ")"))

    head_order = list(range(1, H, 2)) + list(range(0, H, 2))  # odd heads first

    with TileContext(nc) as tc:
        with (
            tc.tile_pool(name="persist", bufs=1) as persist,
            tc.tile_pool(name="acts", bufs=1) as acts,
            tc.tile_pool(name="wpool", bufs=2) as wpool,
            tc.tile_pool(name="stage", bufs=3) as stage,
            tc.tile_pool(name="small", bufs=4) as small,
            tc.tile_pool(name="psum", bufs=2, space="PSUM") as psum,
            tc.tile_pool(name="dramcc", bufs=2, space="DRAM") as dcc,
        ):
            # ---- persistent tiles ----
            h_sb = persist.tile([128, ECH, T], F32, tag="h")
            nc.sync.dma_start(h_sb[:], h0_p.ap().rearrange("(q p) t -> p q t", p=128))
            masks_sb = persist.tile([128, 2, 128], BF16, tag="masks")
            nc.sync.dma_start(masks_sb[:], masks_p.ap().rearrange("two p m -> p two m"))
            ones_bf = persist.tile([128, 128], BF16, tag="ones_bf")
            nc.gpsimd.memset(ones_bf[:], 1.0)
            cInvE = persist.tile([1, 128], F32, tag="cInvE")
            nc.gpsimd.memset(cInvE[:], 1.0 / E)
            cE = persist.tile([1, 128], F32, tag="cE")
            nc.gpsimd.memset(cE[:], float(E))

            # activations
            aT_own = acts.tile([128, ECH, T], BF16, tag="aT_own")
            aT_peer = acts.tile([128, ECH, T], BF16, tag="aT_peer")
            qT = acts.tile([128, ECH, T], BF16, tag="qT")
            kT_own = acts.tile([128, ECH, T], BF16, tag="kT_own")
            kT_peer = acts.tile([128, ECH, T], BF16, tag="kT_peer")
            v_own = acts.tile([128, CH, H, 65], BF16, tag="v_own")
            v_peer = acts.tile([128, CH, H, 65], BF16, tag="v_peer")
            yT_c = acts.tile([128, ECH, T], BF16, tag="yT_c")
            mT = acts.tile([128, ECH, T], BF16, tag="mT")
            gT = acts.tile([128, FCH, T], BF16, tag="gT")
            # ones columns of v (never overwritten by the copies)
            nc.gpsimd.memset(v_own[:, :, :, 64:65], 1.0)
            nc.gpsimd.memset(v_peer[:, :, :, 64:65], 1.0)

            pid = nc.sync.partition_id()
            peer_idx = 1 - (pid % 2)

            def ln_fm(dst):
                """Feature-major layernorm of h_sb -> dst (bf16 [128, ECH, T])."""
                h2cs = [stage.tile([128, T], BF16, tag="h2c", name=f"h2c{i}", bufs=2)
                        for i in range(ECH)]
                hbcs = [stage.tile([128, T], BF16, tag="hbc", name=f"hbc{i}", bufs=2)
                        for i in range(ECH)]
                for kc in range(ECH):
                    nc.scalar.activation(h2cs[kc][:], h_sb[:, kc, :], AF.Square)
                    nc.vector.tensor_copy(out=hbcs[kc][:], in_=h_sb[:, kc, :])
                sum_ps = psum.tile([1, T], F32, tag="y", bufs=2)
                for kc in range(ECH):
                    nc.tensor.matmul(sum_ps[:], ones_bf[:, 0:1], hbcs[kc][:],
                                     start=(kc == 0), stop=(kc == ECH - 1))
                sq_ps = psum.tile([1, T], F32, tag="score")
                for kc in range(ECH):
                    nc.tensor.matmul(sq_ps[:], ones_bf[:, 0:1], h2cs[kc][:],
                                     start=(kc == 0), stop=(kc == ECH - 1))
                s1 = small.tile([1, T], F32, tag="s1", bufs=1)
                nc.vector.tensor_copy(out=s1[:], in_=sum_ps[:])
                t1 = small.tile([1, T], F32, tag="lnt", bufs=3)
                nc.vector.tensor_tensor(t1[:], s1[:], s1[:], ALU.mult)
                t2 = small.tile([1, T], F32, tag="lnt", bufs=3)
                nc.vector.tensor_scalar(t2[:], sq_ps[:], float(E), float(E) * E * EPS,
                                        ALU.mult, ALU.add)
                dmy1 = psum.tile([1, 1], F32, tag="mm")
                nc.tensor.matmul(dmy1[:], t2[:, 0:1], cInvE[:, 0:1],
                                 start=True, stop=True)  # HAM keep-alive
                t3 = small.tile([1, T], F32, tag="lnt", bufs=3)
                nc.vector.tensor_tensor(t3[:], t2[:], t1[:], ALU.subtract)
                t4 = small.tile([1, T], F32, tag="lnt", bufs=3)
                nc.scalar.activation(t4[:], t3[:], AF.Sqrt)
                r = small.tile([1, T], F32, tag="r", bufs=1)
                nc.vector.reciprocal_approx_fast(out=r[:], in_=t4[:])
                dmy2 = psum.tile([1, 1], F32, tag="mm")
                nc.tensor.matmul(dmy2[:], t4[:, 0:1], cInvE[:, 0:1],
                                 start=True, stop=True)  # HAM keep-alive
                # broadcast mean = s1/E and rstd = E*r across partitions via PE
                mb_ps = psum.tile([128, T], F32, tag="score")
                nc.tensor.matmul(mb_ps[:], cInvE[:], s1[:], start=True, stop=True)
                rb_ps = psum.tile([128, T], F32, tag="score")
                nc.tensor.matmul(rb_ps[:], cE[:], r[:], start=True, stop=True)
                mb = stage.tile([128, T], F32, tag="mb", bufs=1)
                nc.scalar.activation(mb[:], mb_ps[:], AF.Copy)
                rb = stage.tile([128, T], F32, tag="rb", bufs=1)
                nc.scalar.activation(rb[:], rb_ps[:], AF.Copy)
                # apply split across DVE and GPSIMD so chunks arrive ~2x faster
                ngps = ECH // 3
                for kc in range(ECH):
                    tmp = stage.tile([128, T], F32, tag="lntmp", bufs=2)
                    eng = nc.gpsimd if kc >= ECH - ngps else nc.vector
                    eng.tensor_tensor(tmp[:], h_sb[:, kc, :], mb[:], ALU.subtract)
                    eng.tensor_tensor(dst[:, kc, :], tmp[:], rb[:], ALU.mult)
                    # dep-spread keep-alive: a trivial matmul reading this chunk
                    # lands on the PE mid-apply, resetting the HAM idle window
                    dmy = psum.tile([1, 1], F32, tag="mm", name=f"lndmy{kc}")
                    nc.tensor.matmul(dmy[:], dst[:, kc, 0:1], ones_bf[:, 0:1],
                                     start=True, stop=True)

            def ship(l):
                """DMA aT_own to DRAM and AllGather with the pair partner."""
                cc_in = dcc.tile([E * T], BF16, tag="cc_in")
                cc_out = dcc.tile([2, E * T], BF16, tag="cc_out")
                src = cc_in[:].rearrange("(p q t) -> p q t", p=128, q=ECH)
                half = max(ECH // 2, 1)
                for g in range(0, ECH, half):
                    g2 = min(g + half, ECH)
                    nc.sync.dma_start(src[:, g:g2, :], aT_own[:, g:g2, :])
                nc.gpsimd.collective_compute(
                    "AllGather", ALU.bypass,
                    replica_groups=[[0, 1], [2, 3], [4, 5], [6, 7]],
                    ins=[cc_in[:]], outs=[cc_out[:]])
                return cc_out

            # layer 0 LN1 + exchange
            ln_fm(aT_own)
            cc_out = ship(0)

            for l in range(L):
                wq = wpool.tile([128, ECH, 3 * E], BF16, tag="W")
                nc.sync.dma_start(wq[:], wqkv_p[l])
                wp = wpool.tile([128, ECH, E], BF16, tag="W")
                nc.sync.dma_start(wp[:], wproj_p[l])
                if has_bias['qkv']:
                    bqk_sb = small.tile([128, 2 * ECH], F32, tag="bqk")
                    nc.sync.dma_start(bqk_sb[:], bqk_p[l].rearrange("c p one -> p (c one)"))
                if has_bias['v']:
                    bv_sb = small.tile([1, E], BF16, tag="bv")
                    nc.sync.dma_start(bv_sb[:], bv_p[l])

                def qk_out(ps, dst, wcol):
                    if has_bias['qkv']:
                        nc.vector.tensor_scalar_add(dst, ps[:], bqk_sb[:, wcol:wcol + 1])
                    else:
                        nc.vector.tensor_copy(out=dst, in_=ps[:])

                # q
                for oc in range(ECH):
                    ps = psum.tile([128, T], F32, tag="mm")
                    for kc in range(ECH):
                        nc.tensor.matmul(ps[:], wq[:, kc, 128 * oc:128 * (oc + 1)],
                                         aT_own[:, kc, :], start=(kc == 0), stop=(kc == ECH - 1))
                    qk_out(ps, qT[:, oc, :], oc)
                # k / v for a given source activation
                def k_chunks(aT, kT):
                    for oc in range(ECH):
                        ps = psum.tile([128, T], F32, tag="mm")
                        for kc in range(ECH):
                            nc.tensor.matmul(ps[:], wq[:, kc, E + 128 * oc:E + 128 * (oc + 1)],
                                             aT[:, kc, :], start=(kc == 0), stop=(kc == ECH - 1))
                        qk_out(ps, kT[:, oc, :], ECH + oc)

                def v_tile(aT, vt, t, nn):
                    ps = psum.tile([128, NW], F32, tag="mm")
                    for kc in range(ECH):
                        nc.tensor.matmul(
                            ps[:], aT[:, kc, 128 * t:128 * (t + 1)],
                            wq[:, kc, 2 * E + nn * NW:2 * E + (nn + 1) * NW],
                            start=(kc == 0), stop=(kc == ECH - 1 and not has_bias['v']))
                    if has_bias['v']:
                        nc.tensor.matmul(ps[:], ones_bf[0:1, 0:128],
                                         bv_sb[0:1, nn * NW:(nn + 1) * NW],
                                         start=False, stop=True)
                    dst = vt[:, t, nn * H2:(nn + 1) * H2, 0:64]
                    nc.vector.tensor_copy(
                        out=dst, in_=ps[:].rearrange("p (h dd) -> p h dd", h=H2))

                k_chunks(aT_own, kT_own)
                for t in range(CH):
                    v_tile(aT_own, v_own, t, 0)

                # peer activations + peer k and v half 0 (the nn=1 halves of
                # both v's are interleaved into attention as dense PE filler)
                nc.sync.dma_start(
                    aT_peer[:],
                    cc_out[peer_idx].rearrange("(p q t) -> p q t", p=128, q=ECH))
                k_chunks(aT_peer, kT_peer)
                for t in range(CH):
                    v_tile(aT_peer, v_peer, t, 0)

                wf = wpool.tile([128, ECH, F], BF16, tag="W")
                nc.sync.dma_start(wf[:], wfc_p[l])

                # ---------------- attention ----------------
                # v halves 1 feed heads >= H2: all CH tiles must be emitted
                # before the first such head in head_order.
                first_big = 0
                for hh in head_order:
                    if hh >= H2:
                        break
                    first_big += 1
                fillers = [(aT_own, v_own, t) for t in range(CH)] + \
                          [(aT_peer, v_peer, t) for t in range(CH)]
                vp1_slots = [[] for _ in range(max(first_big, 1))]
                for j, f in enumerate(fillers):
                    vp1_slots[j % max(first_big, 1)].append(f)
                if first_big == 0:
                    for aTf, vtf, tf in fillers:
                        v_tile(aTf, vtf, tf, 1)
                    vp1_slots = []

                def att_head(hh, yps):
                    plo, po = 64 * (hh % 2), hh // 2

                    def avs(cc, att):
                        # cc=0: full-range matmuls (start=True clears the whole
                        # bank on HW, so exactly one start covering [0,T)).
                        # cc>0: unmasked tail first, masked diagonal separate —
                        # all pure accumulation.
                        if cc == 0:
                            for par, vt in ((0, v_own), (1, v_peer)):
                                nc.tensor.matmul(yps[:, 0:T], vt[:, cc, hh, :],
                                                 att[:, par, 0:T],
                                                 start=(par == 0),
                                                 stop=(par == 1 and CH == 1),
                                                 skip_group_check=True)
                            return
                        NR = T - 128 * (cc + 1)
                        for par, vt in ((0, v_own), (1, v_peer)):
                            if NR > 0:
                                nc.tensor.matmul(yps[:, 128 * (cc + 1):T],
                                                 vt[:, cc, hh, :],
                                                 att[:, par, 128:128 + NR],
                                                 start=False, stop=False,
                                                 skip_group_check=True)
                        for par, vt in ((0, v_own), (1, v_peer)):
                            nc.tensor.matmul(yps[:, 128 * cc:128 * (cc + 1)],
                                             vt[:, cc, hh, :], att[:, par, 0:128],
                                             start=False, stop=(
                                                 par == 1 and cc == CH - 1),
                                             skip_group_check=True)

                    atts = []
                    for cc in range(CH):
                        N = T - 128 * cc
                        aps = psum.tile([128, 2, T], F32, tag="score")
                        nc.tensor.matmul(aps[:, 0, 0:N],
                                         kT_own[plo:plo + 64, po, 128 * cc:128 * (cc + 1)],
                                         qT[plo:plo + 64, po, 128 * cc:T],
                                         start=True, stop=True)
                        nc.tensor.matmul(aps[:, 1, 0:N],
                                         kT_peer[plo:plo + 64, po, 128 * cc:128 * (cc + 1)],
                                         qT[plo:plo + 64, po, 128 * cc:T],
                                         start=True, stop=True)
                        att = stage.tile([128, 2, T], BF16, tag="att")
                        nc.scalar.activation(att[:, :, 0:N], aps[:, :, 0:N], AF.Exp)
                        nc.vector.tensor_tensor(att[:, 0, 0:128], att[:, 0, 0:128],
                                                masks_sb[:, 0, :], ALU.mult)
                        nc.gpsimd.tensor_tensor(att[:, 1, 0:128], att[:, 1, 0:128],
                                                masks_sb[:, 1, :], ALU.mult)
                        atts.append(att)
                        if cc > 0:
                            avs(cc - 1, atts[cc - 1])
                    avs(CH - 1, atts[CH - 1])

                def att_norm(hh, yps):
                    po = hh // 2
                    # denominators row -> bf16 -> rank-1 broadcast -> reciprocal
                    dbf = small.tile([128, T], BF16, tag="dbf", bufs=2)
                    nc.vector.tensor_copy(out=dbf[64:65, :], in_=yps[64:65, :])
                    bps = psum.tile([128, T], F32, tag="mm")
                    nc.tensor.matmul(bps[0:64, :], ones_bf[64:65, 0:64],
                                     dbf[64:65, :], start=True, stop=True)
                    rb = stage.tile([64, T], F32, tag="bc", bufs=2)
                    nc.vector.reciprocal_approx_fast(out=rb[:], in_=bps[0:64, :])
                    dst = yT_c[0:64, po, :] if hh % 2 == 0 else mT[0:64, po, :]
                    nc.vector.tensor_tensor(dst, yps[0:64, :], rb[:], ALU.mult)

                for i, hh in enumerate(head_order):
                    yps = psum.tile([65, T], F32, tag="y", bufs=2)
                    att_head(hh, yps)
                    att_norm(hh, yps)
                    if i < len(vp1_slots):  # dense PE filler between early heads
                        for aTf, vtf, tf in vp1_slots[i]:
                            v_tile(aTf, vtf, tf, 1)
                nc.sync.dma_start(yT_c[64:128, :, :], mT[0:64, :, :])

                if l == 0:
                    do_dump("aT", aT_own); do_dump("qT", qT)
                    do_dump("kTo", kT_own); do_dump("kTp", kT_peer)
                    do_dump("vo", v_own); do_dump("vp", v_peer)
                    do_dump("yTc", yT_c)

                # ---------------- proj + residual ----------------
                if has_bias['proj']:
                    bproj_sb = small.tile([128, ECH], F32, tag="bproj")
                    nc.sync.dma_start(bproj_sb[:], bproj_p[l].rearrange("c p one -> p (c one)"))
                for oc in range(ECH):
                    ps = psum.tile([128, T], F32, tag="mm")
                    for kc in range(ECH):
                        nc.tensor.matmul(ps[:], wp[:, kc, 128 * oc:128 * (oc + 1)],
                                         yT_c[:, kc, :], start=(kc == 0), stop=(kc == ECH - 1))
                    hs = h_sb[:, oc, :]
                    nc.vector.tensor_tensor(hs, hs, ps[:], ALU.add)
                    if has_bias['proj']:
                        nc.vector.tensor_scalar_add(hs, hs, bproj_sb[:, oc:oc + 1])

                wf2 = wpool.tile([128, FCH, E], BF16, tag="W")
                nc.sync.dma_start(wf2[:], wfc2_p[l])
                if l == 0:
                    do_dump("h1", h_sb)

                # ---------------- ln2 + fc1 + fc2 ----------------
                ln_fm(mT)
                if has_bias['fc']:
                    bfc_sb = small.tile([128, FCH], F32, tag="bfc")
                    nc.sync.dma_start(bfc_sb[:], bfc_p[l].rearrange("c p one -> p (c one)"))
                for fm in range(FCH):
                    ps = psum.tile([128, T], F32, tag="mm")
                    for kc in range(ECH):
                        nc.tensor.matmul(ps[:], wf[:, kc, 128 * fm:128 * (fm + 1)],
                                         mT[:, kc, :], start=(kc == 0), stop=(kc == ECH - 1))
                    bias_arg = bfc_sb[:, fm:fm + 1] if has_bias['fc'] else 0.0
                    nc.scalar.activation(gT[:, fm, :], ps[:], AF.Gelu_apprx_tanh, bias=bias_arg)
                if l == 0:
                    do_dump("gT", gT)

                if has_bias['fc2']:
                    bfc2_sb = small.tile([128, ECH], F32, tag="bfc2")
                    nc.sync.dma_start(bfc2_sb[:], bfc2_p[l].rearrange("c p one -> p (c one)"))
                for oc in range(ECH):
                    ps = psum.tile([128, T], F32, tag="mm")
                    for kc in range(FCH):
                        nc.tensor.matmul(ps[:], wf2[:, kc, 128 * oc:128 * (oc + 1)],
                                         gT[:, kc, :], start=(kc == 0), stop=(kc == FCH - 1))
                    hs = h_sb[:, oc, :]
                    nc.vector.tensor_tensor(hs, hs, ps[:], ALU.add)
                    if has_bias['fc2']:
                        nc.vector.tensor_scalar_add(hs, hs, bfc2_sb[:, oc:oc + 1])
                if l == 0:
                    do_dump("h2", h_sb)

                # next layer's LN1 + exchange (or final LN)
                ln_fm(aT_own)
                if l < L - 1:
                    cc_out = ship(l + 1)

            # ---------------- lm head ----------------
            if has_bias['lm']:
                blm_sb = small.tile([1, VNC * 512], BF16, tag="blm")
                nc.sync.dma_start(blm_sb[:], blm_p[:])
            lm_tags = ["mm", "mm", "score", "score", "y"]
            wl = None
            for n in range(VNC):
                if n % 2 == 0:
                    wl = wpool.tile([128, ECH, 1024], BF16, tag="W")
                    ncols = min(1024, (VNC - n) * 512)
                    nc.sync.dma_start(wl[:, :, 0:ncols],
                                      wlm_p[:, :, 512 * n:512 * n + ncols])
                wo = (n % 2) * 512
                lout = stage.tile([128, CH, 512], BF16, tag="lout", bufs=2)
                for t in range(CH):
                    ps = psum.tile([128, 512], F32, tag=lm_tags[(n * CH + t) % len(lm_tags)],
                                   name=f"lmps{t}")
                    for kc in range(ECH):
                        nc.tensor.matmul(ps[:], aT_own[:, kc, 128 * t:128 * (t + 1)],
                                         wl[:, kc, wo:wo + 512],
                                         start=(kc == 0), stop=(kc == ECH - 1 and not has_bias['lm']))
                    if has_bias['lm']:
                        nc.tensor.matmul(ps[:], ones_bf[0:1, 0:128],
                                         blm_sb[0:1, 512 * n:512 * (n + 1)],
                                         start=False, stop=True)
                    if t % 2 == 0:
                        nc.vector.tensor_copy(out=lout[:, t, :], in_=ps[:])
                    else:
                        nc.scalar.activation(lout[:, t, :], ps[:], AF.Copy)
                nc.sync.dma_start(
                    out_p.ap().rearrange("(t p) v -> p t v", p=128)[:, :, 512 * n:512 * (n + 1)],
                    lout[:])
    return nc


# ---------------------------------------------------------------------------
# host prep
# ---------------------------------------------------------------------------

def host_prep(inputs, c):
    d = derived(c)
    B, S, L, H, D, F, V = c['B'], c['S'], c['L'], c['H'], c['D'], c['F'], c['V']
    E, T, ECH, FCH, VNC = d['E'], d['T'], d['ECH'], d['FCH'], d['VNC']

    f32 = lambda a: np.asarray(a, np.float32)
    x = np.asarray(inputs['x']).astype(np.int64)
    wte, wpe = f32(inputs['wte']), f32(inputs['wpe'])
    g1, b1 = f32(inputs['ln1_g']), f32(inputs['ln1_b'])
    aw, ab = f32(inputs['attn_w']), f32(inputs['attn_b'])
    pw, pb = f32(inputs['attn_proj_w']), f32(inputs['attn_proj_b'])
    g2, b2 = f32(inputs['ln2_g']), f32(inputs['ln2_b'])
    fw, fb = f32(inputs['fc_w']), f32(inputs['fc_b'])
    p2w, p2b = f32(inputs['fc_proj_w']), f32(inputs['fc_proj_b'])
    gf, bf_ = f32(inputs['lnf_g']), f32(inputs['lnf_b'])
    lm = f32(inputs['lm_head_w'])

    scale = 1.0 / np.sqrt(D)
    aw_f = aw * g1[:, :, None]              # fold ln1 gamma
    ab_f = ab + np.einsum('le,lef->lf', b1, aw)
    aw_f[:, :, :E] *= scale
    ab_f[:, :E] *= scale
    fw_f = fw * g2[:, :, None]
    fb_f = fb + np.einsum('le,lef->lf', b2, fw)
    lm_f = lm * gf[:, None]
    blm_f = bf_ @ lm

    def bfc16(a):
        return np.ascontiguousarray(a).astype(BF)

    wqkv = bfc16(aw_f.reshape(L, ECH, 128, 3 * E).transpose(0, 2, 1, 3))
    wproj = bfc16(pw.reshape(L, ECH, 128, E).transpose(0, 2, 1, 3))
    wfc = bfc16(fw_f.reshape(L, ECH, 128, F).transpose(0, 2, 1, 3))
    wfc2 = bfc16(p2w.reshape(L, FCH, 128, E).transpose(0, 2, 1, 3))
    wlm_pad = np.zeros((E, VNC * 512), np.float32)
    wlm_pad[:, :V] = lm_f
    wlm = bfc16(wlm_pad.reshape(ECH, 128, VNC * 512).transpose(1, 0, 2))

    has_bias = dict(
        qkv=bool(np.any(ab_f[:, :2 * E])), v=bool(np.any(ab_f[:, 2 * E:])),
        proj=bool(np.any(pb)), fc=bool(np.any(fb_f)), fc2=bool(np.any(p2b)),
        lm=bool(np.any(blm_f)))

    # masks [2, 128, 128]: [own, peer]; rows = key slot, cols = query slot in
    # the diagonal 128-block. own: key u <= query u (inclusive). peer for even
    # core: strict; for odd core: inclusive.
    k_ = np.arange(128)[:, None]
    u_ = np.arange(128)[None, :]
    m_own = (k_ <= u_).astype(BF)
    emb = wte[x] + wpe[:S][None, :, :]       # [B, S, E] f32
    in_maps = []
    metas = []
    for core in range(8):
        b, p = core // 2, core % 2
        h0 = np.ascontiguousarray(emb[b, p::2, :].T).astype(np.float32)  # [E, T]
        m_peer = ((k_ < u_) if p == 0 else (k_ <= u_)).astype(BF)
        m = dict(h0=h0, wqkv=wqkv, wproj=wproj, wfc=wfc, wfc2=wfc2, wlm=wlm,
                 masks=np.stack([m_own, m_peer]))
        if has_bias['qkv']:
            m['bqk'] = np.ascontiguousarray(
                ab_f[:, :2 * E].reshape(L, 2 * ECH, 128, 1)).astype(np.float32)
        if has_bias['v']:
            m['bv'] = ab_f[:, 2 * E:].reshape(L, 1, E).astype(BF)
        if has_bias['proj']:
            m['bproj'] = np.ascontiguousarray(pb.reshape(L, ECH, 128, 1)).astype(np.float32)
        if has_bias['fc']:
            m['bfc'] = np.ascontiguousarray(fb_f.reshape(L, FCH, 128, 1)).astype(np.float32)
        if has_bias['fc2']:
            m['bfc2'] = np.ascontiguousarray(p2b.reshape(L, ECH, 128, 1)).astype(np.float32)
        if has_bias['lm']:
            blm_pad = np.zeros((1, VNC * 512), np.float32)
            blm_pad[0, :V] = blm_f
            m['blm'] = blm_pad.astype(BF)
        in_maps.append(m)
        metas.append((b, p))
    return in_maps, metas, has_bias


def run(inputs, c, nc=None, has_bias=None, in_maps=None, metas=None, dump=(), want_raw=False, trace=False):
    if in_maps is None:
        in_maps, metas, has_bias = host_prep(inputs, c)
    if nc is None:
        nc = build(c, has_bias, dump=dump)
        nc.compile()
    res = run_bass_kernel_spmd(nc, in_maps, core_ids=list(range(8)), trace=trace)
    d = derived(c)
    B, S, V = c['B'], c['S'], c['V']
    out = np.empty((B, S, V), np.float32)
    for core in range(8):
        b, p = metas[core]
        out[b, p::2, :] = np.asarray(res.results[core]["logits"])[:, :V].astype(np.float32)
    if want_raw:
        return out, nc, res
    return out, nc


# ---------------------------------------------------------------------------
# harness entry point: kernel(**inputs) -> full logits [B, S, V] float32
# ---------------------------------------------------------------------------
_NC_CACHE = {}


def kernel(**inputs):
    c = cfg_full()
    in_maps, metas, has_bias = host_prep(inputs, c)
    key = tuple(sorted(has_bias.items()))
    if key not in _NC_CACHE:
        nc = build(c, has_bias)
        nc.compile()
        _NC_CACHE[key] = nc
    nc = _NC_CACHE[key]
    res = run_bass_kernel_spmd(nc, in_maps, core_ids=list(range(8)))
    d = derived(c)
    B, S, V = c['B'], c['S'], c['V']
    out = np.empty((B, S, V), np.float32)
    for core in range(8):
        b, p = metas[core]
        out[b, p::2, :] = np.asarray(res.results[core]["logits"])[:, :V].astype(np.float32)
    return out
